# Initial kernel scaffold
#
"""Trainium2 Bass kernel for GNN message passing (GAT-style attention, L=3 layers).

Strategy (dst-sharded across 8 cores):
  - Attention score decomposed per-node: score[e,h] = s_src[src_e,h] + s_dst[dst_e,h]
    with s_src = h @ (Wv . a_v), s_dst = h @ (Wq . a_q) + K*c1 + c0  (host-folded weights).
  - Softmax without max-subtraction (scores are O(1)); normalization folded to
    per-node divide after aggregation: agg = (sum ex*v) / (sum ex).
  - h lives in a replicated DRAM table with rows [bf16_hi | bf16_lo] (exact-ish f32).
    Per-edge h[src] fetched by transposed dma_gather -> feature-major SBUF, feeding
    v = h.T @ Wv matmuls per 128-edge slot.
  - Segment-sum by dst via 0/1 indicator matmuls into PSUM (edges pre-sorted by dst
    on host; each batch = one 128-node tile).
  - Per-tile fused dense update: normalize, Wo matmul (via PE transpose), residual,
    LN, gate, relu; AllGather of each core's h-shard rebuilds the table per layer.
"""

import os
import sys
import numpy as np

sys.path.insert(0, "/opt/trn_rl_repo")

import ml_dtypes  # noqa: E402

# ---- problem constants (hardcoded per contract) ----
N, E, D, H, L = 50000, 400000, 128, 4, 3
IN_DIM, OUT_DIM = 128, 64
NEG_SLOPE = 0.01
LN_EPS = 1e-5

NCORES = 8
SHARD = N // NCORES            # 6250
NT = (SHARD + 127) // 128      # 49 tiles per shard
SHARD_PAD = NT * 128           # 6272
NTAB = NCORES * SHARD_PAD      # 50176 table rows
CHUNK = 32768                  # int16 gather index limit
PLANES = int(os.environ.get("GNN_PLANES", "2"))  # 2 = hi/lo f32-ish, 1 = bf16 only
ROW = PLANES * 128             # bf16 words per table row
SENT_ROW = SHARD_PAD           # sentinel row in s_dst table (value -1e30)
SD_ROWS = SHARD_PAD + 128      # s_dst table rows

_CACHE = {}


def _trow(n):
    """global node id -> table row"""
    return (n // SHARD) * SHARD_PAD + (n % SHARD)


def _wrap_idx(a):
    """flat int16 idx list (len mult of 16) -> [128, len//16] wrapped+replicated"""
    a = np.asarray(a, np.int16)
    w = a.reshape(-1, 16).T  # [16, n/16]
    return np.tile(w, (8, 1))


def _prep(inputs):
    """Host-side preprocessing: fold weights, build per-core edge batches."""
    f32 = np.float32
    x = np.asarray(inputs["x"], f32)
    ei = np.asarray(inputs["edge_index"], np.int64)
    K = np.asarray(inputs["K"], f32)
    W_in = np.asarray(inputs["W_in"], f32)
    b_in = np.asarray(inputs["b_in"], f32)
    Wk_w = np.asarray(inputs["Wk_w"], f32)
    Wk_b = np.asarray(inputs["Wk_b"], f32)
    Wq = np.asarray(inputs["Wq"], f32)
    Wv = np.asarray(inputs["Wv"], f32)
    att = np.asarray(inputs["att"], f32)
    Wo = np.asarray(inputs["Wo"], f32)
    b_l = np.asarray(inputs["b_l"], f32)
    ln_g = np.asarray(inputs["ln_g"], f32)
    ln_b = np.asarray(inputs["ln_b"], f32)
    gate_w = np.asarray(inputs["gate_w"], f32)
    gate_b = np.asarray(inputs["gate_b"], f32)
    W_out = np.asarray(inputs["W_out"], f32)
    b_out = np.asarray(inputs["b_out"], f32)

    src, dst = ei[0], ei[1]

    # ---- folded weights ----
    a_k, a_q, a_v = att[:, :, :D], att[:, :, D:2 * D], att[:, :, 2 * D:]
    # Wqa[l][k,h] = sum_d Wq[l][k, h*D+d] * a_q[l][h,d]
    Wq_r = Wq.reshape(L, D, H, D)
    Wv_r = Wv.reshape(L, D, H, D)
    Wqa = np.einsum("lkhd,lhd->lkh", Wq_r, a_q).astype(f32)   # [L,128,4]
    Wva = np.einsum("lkhd,lhd->lkh", Wv_r, a_v).astype(f32)   # [L,128,4]
    c1 = np.einsum("ld,lhd->lh", Wk_w, a_k).astype(f32)       # [L,4]
    c0 = np.einsum("ld,lhd->lh", Wk_b, a_k).astype(f32)       # [L,4]
    gv1 = np.einsum("ld,lde->le", Wk_w, gate_w).astype(f32)   # [L,128]
    gv0 = (np.einsum("ld,lde->le", Wk_b, gate_w) + gate_b).astype(f32)

    def bc(v, reps=128):  # broadcast a free-axis vector across partitions
        return np.tile(np.asarray(v, f32)[None, :], (reps, 1))

    meta = {}
    common = {
        "Win": W_in.copy(),
        "binb": bc(b_in),
        "Wv3": Wv.copy(),                                    # [3,128,512]
        "Wqa3": Wqa, "Wva3": Wva,
        "c1b3": np.stack([bc(c1[l]) for l in range(L)]),     # [3,128,4]
        "c0b3": np.stack([bc(c0[l]) for l in range(L)]),
        "Wo3": Wo.reshape(L, 4, 128, 128).copy(),            # chunk kc = rows kc*128..
        "lngb3": np.stack([bc(ln_g[l]) for l in range(L)]),
        "lnbb3": np.stack([bc(ln_b[l]) for l in range(L)]),
        "blb3": np.stack([bc(b_l[l]) for l in range(L)]),
        "gv1b3": np.stack([bc(gv1[l]) for l in range(L)]),
        "gv0b3": np.stack([bc(gv0[l]) for l in range(L)]),
        "Wout": W_out.copy(),
        "boutb": bc(b_out),
        "iota": np.tile(np.arange(128, dtype=f32)[None, :], (128, 1)),
        "ident": np.eye(128, dtype=f32),
    }

    # xT: [128, NTAB] feature-major padded table layout
    xpad = np.zeros((NTAB, D), f32)
    for s in range(NCORES):
        xpad[s * SHARD_PAD:s * SHARD_PAD + SHARD] = x[s * SHARD:(s + 1) * SHARD]
    common["xT"] = np.ascontiguousarray(xpad.T)

    # ---- per-core edge batches ----
    tr_src = _trow(src)                       # table row of each edge's src
    core_of = dst // SHARD
    ld_all = dst % SHARD

    per_core = [{} for _ in range(NCORES)]
    # per core, per tile: (low_rows, high_rows, r_low, r_high, sd_low, sd_high)
    tiles = [[None] * NT for _ in range(NCORES)]
    k1r = np.zeros((NCORES, NT), np.int64)
    k2r = np.zeros((NCORES, NT), np.int64)
    for c in range(NCORES):
        m = core_of == c
        sc, lc = tr_src[m], ld_all[m]
        order = np.argsort(lc, kind="stable")
        sc, lc = sc[order], lc[order]
        tile_id = lc // 128
        bounds = np.searchsorted(tile_id, np.arange(NT + 1))
        for t in range(NT):
            s_t = sc[bounds[t]:bounds[t + 1]]
            l_t = lc[bounds[t]:bounds[t + 1]]
            low = s_t < CHUNK
            tiles[c][t] = (s_t[low], s_t[~low] - CHUNK, l_t[low] % 128,
                           l_t[~low] % 128, l_t[low], l_t[~low])
            k1r[c, t] = low.sum()
            k2r[c, t] = (~low).sum()

    K1 = (np.ceil(k1r.max(axis=0) / 128).astype(np.int64) * 128)
    K2 = (np.ceil(k2r.max(axis=0) / 128).astype(np.int64) * 128)
    C = (K1 + K2) // 128
    meta["K1"], meta["K2"], meta["C"] = K1.tolist(), K2.tolist(), C.tolist()
    S = int(C.sum())
    meta["S"] = S

    for c in range(NCORES):
        src_cols, sd_cols, rl = [], [], np.full((128, S), 200.0, f32)
        soff = 0
        for t in range(NT):
            lo_s, hi_s, lo_r, hi_r, lo_sd, hi_sd = tiles[c][t]
            k1, k2 = int(K1[t]), int(K2[t])
            # low segment then high segment; pads -> row 0 / sentinel
            seg_src = np.zeros(k1 + k2, np.int64)
            seg_src[:len(lo_s)] = lo_s
            seg_src[k1:k1 + len(hi_s)] = hi_s
            seg_sd = np.full(k1 + k2, SENT_ROW, np.int64)
            seg_sd[:len(lo_sd)] = lo_sd
            seg_sd[k1:k1 + len(hi_sd)] = hi_sd
            seg_r = np.full(k1 + k2, 200.0, f32)
            seg_r[:len(lo_r)] = lo_r
            seg_r[k1:k1 + len(hi_r)] = hi_r
            if k1:
                src_cols.append(_wrap_idx(seg_src[:k1]))
            if k2:
                src_cols.append(_wrap_idx(seg_src[k1:]))
            sd_cols.append(_wrap_idx(seg_sd))
            ct = (k1 + k2) // 128
            rl[:, soff:soff + ct] = seg_r.reshape(ct, 128).T
            soff += ct
        d = per_core[c]
        d["srcidx"] = np.concatenate(src_cols, axis=1) if src_cols else np.zeros((128, 16), np.int16)
        d["sdidx"] = np.concatenate(sd_cols, axis=1)
        d["rloc"] = rl
        kk = np.zeros(SHARD_PAD, f32)
        kk[:SHARD] = K[c * SHARD:(c + 1) * SHARD]
        d["Ksb"] = np.ascontiguousarray(kk.reshape(NT, 128).T)  # [128, NT]
        d.update(common)
    meta["src_cols_total"] = per_core[0]["srcidx"].shape[1]
    meta["sd_cols_total"] = per_core[0]["sdidx"].shape[1]
    return meta, per_core


def _build(meta):
    import concourse.bass as bass
    import concourse.tile as tile
    from concourse import bacc, mybir

    f32, bf16, i16 = mybir.dt.float32, mybir.dt.bfloat16, mybir.dt.int16
    AOP, AF = mybir.AluOpType, mybir.ActivationFunctionType
    K1, K2, C, S = meta["K1"], meta["K2"], meta["C"], meta["S"]
    CMAX = max(C)
    K1MAX, K2MAX = max(K1), max(K2)

    nc = bacc.Bacc("TRN2", target_bir_lowering=False, debug=False,
                   num_devices=NCORES)

    # ---- I/O ----
    def inp(name, shape, dt=f32):
        return nc.dram_tensor(name, list(shape), dt, kind="ExternalInput")

    xT = inp("xT", [128, NTAB])
    Ksb_t = inp("Ksb", [128, NT])
    Win_t = inp("Win", [128, 128]); binb_t = inp("binb", [128, 128])
    Wv3_t = inp("Wv3", [L, 128, 512])
    Wqa3_t = inp("Wqa3", [L, 128, 4]); Wva3_t = inp("Wva3", [L, 128, 4])
    c1b3_t = inp("c1b3", [L, 128, 4]); c0b3_t = inp("c0b3", [L, 128, 4])
    Wo3_t = inp("Wo3", [L, 4, 128, 128])
    lngb3_t = inp("lngb3", [L, 128, 128]); lnbb3_t = inp("lnbb3", [L, 128, 128])
    blb3_t = inp("blb3", [L, 128, 128])
    gv1b3_t = inp("gv1b3", [L, 128, 128]); gv0b3_t = inp("gv0b3", [L, 128, 128])
    Wout_t = inp("Wout", [128, 64]); boutb_t = inp("boutb", [128, 64])
    iota_t = inp("iota", [128, 128]); ident_t = inp("ident", [128, 128])
    srcidx_t = inp("srcidx", [128, meta["src_cols_total"]], i16)
    sdidx_t = inp("sdidx", [128, meta["sd_cols_total"]], i16)
    rloc_t = inp("rloc", [128, S])
    out_t = nc.dram_tensor("out", [SHARD_PAD, 64], f32, kind="ExternalOutput")

    with tile.TileContext(nc) as tc:
        dram = tc.tile_pool(name="dram", bufs=1, space="DRAM").__enter__()
        tabs = [dram.tile([NTAB, ROW], bf16, tag=f"tab{i}") for i in range(L)]
        shblks = [dram.tile([SHARD_PAD, ROW], bf16, tag=f"shblk{i}") for i in range(2)]
        sdsts = [dram.tile([SD_ROWS, 64], f32, tag=f"sdst{i}") for i in range(L)]

        pers = tc.tile_pool(name="pers", bufs=1).__enter__()
        wk = tc.tile_pool(name="wk", bufs=2).__enter__()
        wk3 = tc.tile_pool(name="wk3", bufs=3).__enter__()
        ps_v = tc.tile_pool(name="ps_v", bufs=2, space="PSUM").__enter__()
        ps_s = tc.tile_pool(name="ps_s", bufs=2, space="PSUM").__enter__()
        ps_u = tc.tile_pool(name="ps_u", bufs=1, space="PSUM").__enter__()
        ps_m = tc.tile_pool(name="ps_m", bufs=1, space="PSUM").__enter__()
        ps_d = tc.tile_pool(name="ps_d", bufs=2, space="PSUM").__enter__()

        # ---- persistent SBUF loads ----
        def load(tn, shape, dt=f32, tag=None):
            t = pers.tile(list(shape), dt, tag=tag or tn.name)
            nc.sync.dma_start(out=t[:], in_=tn[tuple(slice(None) for _ in shape)])
            return t

        Win = load(Win_t, [128, 128]); binb = load(binb_t, [128, 128])
        Wout = load(Wout_t, [128, 64]); boutb = load(boutb_t, [128, 64])
        iota = load(iota_t, [128, 128]); ident = load(ident_t, [128, 128])
        Ksb = load(Ksb_t, [128, NT])
        srcidx = load(srcidx_t, [128, meta["src_cols_total"]], i16)
        sdidx = load(sdidx_t, [128, meta["sd_cols_total"]], i16)
        rloc = load(rloc_t, [128, S])
        Wv_l, Wqa_l, Wva_l, c1_l, c0_l = [], [], [], [], []
        Wo_l, lng_l, lnb_l, bl_l, gv1_l, gv0_l = [], [], [], [], [], []
        for l in range(L):
            def ld(tn, shape, tag):
                t = pers.tile(list(shape), f32, tag=f"{tag}{l}")
                nc.sync.dma_start(out=t[:], in_=tn[l])
                return t
            Wv_l.append(ld(Wv3_t, [128, 512], "Wv"))
            Wqa_l.append(ld(Wqa3_t, [128, 4], "Wqa"))
            Wva_l.append(ld(Wva3_t, [128, 4], "Wva"))
            c1_l.append(ld(c1b3_t, [128, 4], "c1"))
            c0_l.append(ld(c0b3_t, [128, 4], "c0"))
            Wo_l.append([ld(Wo3_t[l], [128, 128], f"Wo{kc}_") if False else None for kc in range(4)])
            Wo_l[l] = []
            for kc in range(4):
                t = pers.tile([128, 128], f32, tag=f"Wo{l}_{kc}")
                nc.sync.dma_start(out=t[:], in_=Wo3_t[l, kc])
                Wo_l[l].append(t)
            lng_l.append(ld(lngb3_t, [128, 128], "lng"))
            lnb_l.append(ld(lnbb3_t, [128, 128], "lnb"))
            bl_l.append(ld(blb3_t, [128, 128], "bl"))
            gv1_l.append(ld(gv1b3_t, [128, 128], "gv1"))
            gv0_l.append(ld(gv0b3_t, [128, 128], "gv0"))

        h_shard = [pers.tile([128, SHARD_PAD], f32, tag=f"hsh{i}") for i in range(2)]

        # sentinel rows of each s_dst table
        sent = pers.tile([128, 64], f32, tag="sent")
        nc.vector.memset(sent[:], -1e30)
        for l in range(L):
            nc.sync.dma_start(out=sdsts[l][SENT_ROW:SENT_ROW + 128, :], in_=sent[:])

        # ---- helper: hi/lo table-row staging from f32 tile ----
        def stage_rows(h_f32):
            stag = wk.tile([128, ROW], bf16, tag="stag")
            nc.vector.tensor_copy(out=stag[:, 0:128], in_=h_f32[:])
            if PLANES == 2:
                nc.vector.tensor_tensor(out=stag[:, 128:256], in0=h_f32[:],
                                        in1=stag[:, 0:128], op=AOP.subtract)
            return stag

        # ---- helper: s_dst row computation + store (own tile t, layer l) ----
        def sdst_store(hT_sb, t, l):
            psd = ps_d.tile([128, 4], f32, tag="dps_sd")
            nc.tensor.matmul(psd[:], lhsT=hT_sb[:], rhs=Wqa_l[l][:],
                             start=True, stop=True)
            sdt = wk.tile([128, 64], f32, tag="sdstag")
            nc.vector.scalar_tensor_tensor(
                out=sdt[:, 0:4], in0=c1_l[l][:], scalar=Ksb[:, t:t + 1],
                in1=psd[:], op0=AOP.mult, op1=AOP.add)
            nc.vector.tensor_tensor(out=sdt[:, 0:4], in0=sdt[:, 0:4],
                                    in1=c0_l[l][:], op=AOP.add)
            nc.sync.dma_start(out=sdsts[l][t * 128:(t + 1) * 128, :], in_=sdt[:])

        # ---- phase 1: replicated input projection -> tab0 (all 392 tiles) ----
        NTT = NTAB // 128
        for tt in range(NTT):
            xt = wk3.tile([128, 128], f32, tag="xt")
            nc.sync.dma_start(out=xt[:], in_=xT[:, tt * 128:(tt + 1) * 128])
            ph = ps_d.tile([128, 128], f32, tag="dps_h0")
            nc.tensor.matmul(ph[:], lhsT=xt[:], rhs=Win[:], start=True, stop=True)
            h0 = wk3.tile([128, 128], f32, tag="h0")
            nc.vector.tensor_tensor(out=h0[:], in0=ph[:], in1=binb[:], op=AOP.add)
            nc.scalar.activation(out=h0[:], in_=h0[:], func=AF.Relu)
            stag = stage_rows(h0)
            nc.sync.dma_start(out=tabs[0][tt * 128:(tt + 1) * 128, :], in_=stag[:])

        # ---- phase 1b: own-shard h0 capture (dynamic row offset) + sdst layer0 ----
        pid = nc.sync.partition_id()
        row0 = pid * SHARD_PAD
        for t in range(NT):
            g = wk.tile([128, ROW], bf16, tag="g1b")
            nc.sync.dma_start(out=g[:],
                              in_=tabs[0][bass.ds(row0 + t * 128, 128), :])
            hs = h_shard[0][:, t * 128:(t + 1) * 128]
            if PLANES == 2:
                nc.vector.tensor_tensor(out=hs, in0=g[:, 0:128], in1=g[:, 128:256],
                                        op=AOP.add)
            else:
                nc.vector.tensor_copy(out=hs, in_=g[:, 0:128])
            ptr = ps_d.tile([128, 128], f32, tag="dps_tp")
            nc.tensor.transpose(ptr[:], hs, ident[:])
            hT = wk.tile([128, 128], f32, tag="hT1b")
            nc.scalar.copy(out=hT[:], in_=ptr[:])
            sdst_store(hT, t, 0)

        # ---- layers ----
        for l in range(L):
            tab = tabs[l]
            h_cur = h_shard[l % 2]
            h_nxt = h_shard[(l + 1) % 2]
            soff = 0   # slot offset into rloc
            co_src = 0  # col offset into srcidx
            co_sd = 0
            for t in range(NT):
                k1, k2, ct = K1[t], K2[t], C[t]
                # -- gathers --
                hsT = wk.tile([128, PLANES, CMAX * 128], bf16, tag="hsT")
                if k1:
                    gl = wk.tile([128, PLANES, K1MAX], bf16, tag="glow")
                    nc.gpsimd.dma_gather(
                        out_ap=gl[:, :, 0:k1], in_ap=tab[0:CHUNK, :],
                        idxs_ap=srcidx[:, co_src:co_src + k1 // 16],
                        num_idxs=k1, num_idxs_reg=k1, elem_size=ROW,
                        transpose=True)
                    co_src += k1 // 16
                if k2:
                    gh = wk.tile([128, PLANES, K2MAX], bf16, tag="ghigh")
                    nc.gpsimd.dma_gather(
                        out_ap=gh[:, :, 0:k2], in_ap=tab[CHUNK:NTAB, :],
                        idxs_ap=srcidx[:, co_src:co_src + k2 // 16],
                        num_idxs=k2, num_idxs_reg=k2, elem_size=ROW,
                        transpose=True)
                    co_src += k2 // 16
                sd = wk.tile([128, CMAX, 64], f32, tag="sd")
                nc.gpsimd.dma_gather(
                    out_ap=sd[:, 0:ct, :], in_ap=sdsts[l][:, :],
                    idxs_ap=sdidx[:, co_sd:co_sd + ct * 8],
                    num_idxs=ct * 128, num_idxs_reg=ct * 128, elem_size=64,
                    transpose=False)
                co_sd += ct * 8
                # -- reconstruct f32 feature-major h_src --
                hsrc = wk.tile([128, CMAX * 128], f32, tag="hsrc")
                if PLANES == 2:
                    if k1:
                        nc.vector.tensor_tensor(out=hsrc[:, 0:k1], in0=gl[:, 0, 0:k1],
                                                in1=gl[:, 1, 0:k1], op=AOP.add)
                    if k2:
                        nc.vector.tensor_tensor(out=hsrc[:, k1:k1 + k2],
                                                in0=gh[:, 0, 0:k2],
                                                in1=gh[:, 1, 0:k2], op=AOP.add)
                else:
                    if k1:
                        nc.vector.tensor_copy(out=hsrc[:, 0:k1], in_=gl[:, 0, 0:k1])
                    if k2:
                        nc.vector.tensor_copy(out=hsrc[:, k1:k1 + k2], in_=gh[:, 0, 0:k2])

                pu = ps_u.tile([128, 512], f32, tag="pu")
                pm = ps_m.tile([128, 4], f32, tag="pm")
                for c in range(ct):
                    sl = hsrc[:, c * 128:(c + 1) * 128]
                    pv = ps_v.tile([128, 512], f32, tag="pv")
                    nc.tensor.matmul(pv[:], lhsT=sl, rhs=Wv_l[l][:],
                                     start=True, stop=True)
                    pss = ps_s.tile([128, 4], f32, tag="pss")
                    nc.tensor.matmul(pss[:], lhsT=sl, rhs=Wva_l[l][:],
                                     start=True, stop=True)
                    # scores -> ex
                    sc = wk3.tile([128, 4], f32, tag="sc")
                    nc.vector.tensor_tensor(out=sc[:], in0=pss[:],
                                            in1=sd[:, c, 0:4], op=AOP.add)
                    ex = wk3.tile([128, 4], f32, tag="ex")
                    nc.scalar.activation(out=ex[:], in_=sc[:], func=AF.Lrelu,
                                         alpha=NEG_SLOPE)
                    nc.scalar.activation(out=ex[:], in_=ex[:], func=AF.Exp)
                    # messages M = v * ex (per head), split DVE/ACT
                    M = wk3.tile([128, 512], f32, tag="M")
                    for hh in range(H):
                        o = M[:, hh * 128:(hh + 1) * 128]
                        i = pv[:, hh * 128:(hh + 1) * 128]
                        s = ex[:, hh:hh + 1]
                        if hh < 2:
                            nc.vector.tensor_scalar(out=o, in0=i, scalar1=s,
                                                    scalar2=None, op0=AOP.mult)
                        else:
                            nc.scalar.mul(out=o, in_=i, mul=s)
                    # indicator
                    ind = wk3.tile([128, 128], f32, tag="ind")
                    nc.gpsimd.tensor_scalar(out=ind[:], in0=iota[:],
                                            scalar1=rloc[:, soff + c:soff + c + 1],
                                            scalar2=None, op0=AOP.is_equal)
                    nc.tensor.matmul(pu[:], lhsT=ind[:], rhs=M[:],
                                     start=(c == 0), stop=(c == ct - 1))
                    nc.tensor.matmul(pm[:], lhsT=ind[:], rhs=ex[:],
                                     start=(c == 0), stop=(c == ct - 1))
                soff += ct

                # -- normalize agg = U / sm --
                smr = wk.tile([128, 4], f32, tag="smr")
                nc.vector.tensor_scalar(out=smr[:], in0=pm[:], scalar1=1e-30,
                                        scalar2=None, op0=AOP.max)
                nc.vector.reciprocal(out=smr[:], in_=smr[:])
                agg = wk.tile([128, 512], f32, tag="agg")
                for hh in range(H):
                    o = agg[:, hh * 128:(hh + 1) * 128]
                    i = pu[:, hh * 128:(hh + 1) * 128]
                    s = smr[:, hh:hh + 1]
                    if hh < 2:
                        nc.vector.tensor_scalar(out=o, in0=i, scalar1=s,
                                                scalar2=None, op0=AOP.mult)
                    else:
                        nc.scalar.mul(out=o, in_=i, mul=s)
                # -- out = aggT @ Wo + h  (via PE transpose chunks) --
                po = ps_d.tile([128, 128], f32, tag="dps_o")
                for kc in range(4):
                    ptr = ps_d.tile([128, 128], f32, tag="dps_tp")
                    nc.tensor.transpose(ptr[:], agg[:, kc * 128:(kc + 1) * 128],
                                        ident[:])
                    aT = wk3.tile([128, 128], f32, tag="aT")
                    nc.scalar.copy(out=aT[:], in_=ptr[:])
                    nc.tensor.matmul(po[:], lhsT=aT[:], rhs=Wo_l[l][kc][:],
                                     start=(kc == 0), stop=(kc == 3))
                res = wk.tile([128, 128], f32, tag="res")
                nc.vector.tensor_tensor(out=res[:], in0=po[:],
                                        in1=h_cur[:, t * 128:(t + 1) * 128],
                                        op=AOP.add)
                # -- LN --
                mu = wk.tile([128, 1], f32, tag="mu")
                nc.vector.tensor_reduce(out=mu[:], in_=res[:],
                                        axis=_ax(), op=AOP.add)
                nc.vector.tensor_scalar(out=mu[:], in0=mu[:], scalar1=1.0 / 128,
                                        scalar2=None, op0=AOP.mult)
                xc = wk.tile([128, 128], f32, tag="xc")
                nc.vector.tensor_scalar(out=xc[:], in0=res[:], scalar1=mu[:],
                                        scalar2=None, op0=AOP.subtract)
                sq = wk.tile([128, 128], f32, tag="sq")
                vsum = wk.tile([128, 1], f32, tag="vsum")
                nc.scalar.activation(out=sq[:], in_=xc[:], func=AF.Square,
                                     accum_out=vsum[:])
                std = wk.tile([128, 1], f32, tag="std")
                nc.scalar.activation(out=std[:], in_=vsum[:], func=AF.Sqrt,
                                     scale=1.0 / 128, bias=LN_EPS)
                rstd = wk.tile([128, 1], f32, tag="rstd")
                nc.vector.reciprocal(out=rstd[:], in_=std[:])
                t1 = wk.tile([128, 128], f32, tag="t1")
                nc.vector.tensor_scalar(out=t1[:], in0=xc[:], scalar1=rstd[:],
                                        scalar2=None, op0=AOP.mult)
                nc.vector.tensor_tensor(out=t1[:], in0=t1[:], in1=lng_l[l][:],
                                        op=AOP.mult)
                nc.vector.tensor_tensor(out=t1[:], in0=t1[:], in1=lnb_l[l][:],
                                        op=AOP.add)
                # -- gate --
                gt = wk.tile([128, 128], f32, tag="gt")
                nc.vector.scalar_tensor_tensor(out=gt[:], in0=gv1_l[l][:],
                                               scalar=Ksb[:, t:t + 1],
                                               in1=gv0_l[l][:],
                                               op0=AOP.mult, op1=AOP.add)
                nc.scalar.activation(out=gt[:], in_=gt[:], func=AF.Sigmoid)
                # h_next = relu(t1*(1+g) + bl) = relu(t1 + t1*g + bl)
                m1 = wk.tile([128, 128], f32, tag="m1")
                nc.vector.tensor_tensor(out=m1[:], in0=t1[:], in1=gt[:],
                                        op=AOP.mult)
                nc.vector.tensor_tensor(out=m1[:], in0=m1[:], in1=t1[:],
                                        op=AOP.add)
                nc.vector.tensor_tensor(out=m1[:], in0=m1[:], in1=bl_l[l][:],
                                        op=AOP.add)
                hn = h_nxt[:, t * 128:(t + 1) * 128]
                nc.scalar.activation(out=hn, in_=m1[:], func=AF.Relu)

                # transpose h_next for sdst (l<2) or final out (l==2)
                ptr2 = ps_d.tile([128, 128], f32, tag="dps_tp")
                nc.tensor.transpose(ptr2[:], hn, ident[:])
                hT = wk3.tile([128, 128], f32, tag="hTn")
                nc.scalar.copy(out=hT[:], in_=ptr2[:])
                if l < L - 1:
                    sdst_store(hT, t, l + 1)
                    stag = stage_rows(
                        _as_f32_view(nc, hn))
                    nc.sync.dma_start(
                        out=shblks[l][t * 128:(t + 1) * 128, :], in_=stag[:])
                else:
                    pf = ps_d.tile([128, 64], f32, tag="dps_f")
                    nc.tensor.matmul(pf[:], lhsT=hT[:], rhs=Wout[:],
                                     start=True, stop=True)
                    ob = wk.tile([128, 64], f32, tag="ob")
                    nc.vector.tensor_tensor(out=ob[:], in0=pf[:], in1=boutb[:],
                                            op=AOP.add)
                    nc.sync.dma_start(out=out_t[t * 128:(t + 1) * 128, :],
                                      in_=ob[:])
            if l < L - 1:
                nc.gpsimd.collective_compute(
                    "AllGather", mybir.AluOpType.bypass,
                    replica_groups=[list(range(NCORES))],
                    ins=[shblks[l][:].opt()],
                    outs=[tabs[l + 1][:].opt()],
                )

    nc.compile()
    return nc


def _ax():
    from concourse import mybir
    return mybir.AxisListType.X


def _as_f32_view(nc, ap):
    return ap


def kernel(**inputs):
    from concourse import bass_utils

    meta, per_core = _prep(inputs)
    key = "k"
    if key not in _CACHE:
        _CACHE[key] = _build(meta)
    nc = _CACHE[key]
    res = bass_utils.run_bass_kernel_spmd(nc, per_core, core_ids=list(range(NCORES)))
    out = np.concatenate(
        [res.results[c]["out"][:SHARD] for c in range(NCORES)], axis=0)
    return out.astype(np.float32)


if __name__ == "__main__":
    sys.path.insert(0, "/root/problem")
    import reference
    inputs = {k: np.asarray(v) for k, v in reference.setup_inputs().items()}
    got = kernel(**inputs)
    exp = np.asarray(reference.reference(**reference.setup_inputs()))
    err = np.linalg.norm(got - exp) / np.linalg.norm(exp)
    print("Relative error:", err)


# revision 15
# speedup vs baseline: 438.7527x; 438.7527x over previous
"""Trainium2 Bass kernel for GNN message passing (GAT-style attention, L=3 layers).

Strategy (dst-sharded across 8 cores):
  - Attention score decomposed per-node: score[e,h] = s_src[src_e,h] + s_dst[dst_e,h]
    with s_src = h @ (Wv . a_v), s_dst = h @ (Wq . a_q) + K*c1 + c0  (host-folded weights).
  - Softmax without max-subtraction (scores are O(1)); normalization folded to a
    per-node divide after aggregation: agg = (sum ex*v) / (sum ex).
  - h lives in a replicated DRAM table with rows [bf16_hi | bf16_lo] (~f32 precision).
    Per-edge h[src] fetched by transposed dma_gather -> feature-major SBUF, feeding
    v = h.T @ Wv matmuls per 128-edge slot.
  - Segment-sum by dst via 0/1 indicator matmuls into PSUM (edges pre-sorted by dst
    on host; each batch = one 128-node dst tile).
  - Per-tile fused dense update: normalize, Wo matmul (via PE transpose), residual,
    LN, gate, relu; AllGather of each core's h-shard rebuilds the table per layer.
"""

import os
import sys
import numpy as np

sys.path.insert(0, "/opt/trn_rl_repo")

import ml_dtypes  # noqa: F401,E402

# ---- problem constants (hardcoded per contract) ----
N, E, D, H, L = 50000, 400000, 128, 4, 3
IN_DIM, OUT_DIM = 128, 64
NEG_SLOPE = 0.01
LN_EPS = 1e-5

NCORES = 8
SHARD = N // NCORES            # 6250
NT = (SHARD + 127) // 128      # 49 tiles per shard
SHARD_PAD = NT * 128           # 6272
NTAB = NCORES * SHARD_PAD      # 50176 table rows
CHUNK = 32768                  # int16 gather index limit
PLANES = int(os.environ.get("GNN_PLANES", "2"))  # 2 = hi/lo (~f32), 1 = bf16 only
ROW = PLANES * 128             # bf16 words per table row
SENT_ROW = SHARD_PAD           # sentinel row in s_dst table (value -1e30)
SD_ROWS = SHARD_PAD + 128      # s_dst table rows

_CACHE = {}


def _trow(n):
    """global node id -> table row"""
    return (n // SHARD) * SHARD_PAD + (n % SHARD)


def _wrap_idx(a):
    """flat int16 idx list (len mult of 16) -> [128, len//16] wrapped+replicated"""
    a = np.asarray(a, np.int16)
    w = a.reshape(-1, 16).T  # [16, n/16]
    return np.tile(w, (8, 1))


def _prep(inputs):
    """Host-side preprocessing: fold weights, build per-core edge batches."""
    f32 = np.float32
    x = np.asarray(inputs["x"], f32)
    ei = np.asarray(inputs["edge_index"], np.int64)
    K = np.asarray(inputs["K"], f32)

    src, dst = ei[0], ei[1]

    # ---- folded weights ----
    Wq = np.asarray(inputs["Wq"], f32)
    Wv = np.asarray(inputs["Wv"], f32)
    att = np.asarray(inputs["att"], f32)
    Wk_w = np.asarray(inputs["Wk_w"], f32)
    Wk_b = np.asarray(inputs["Wk_b"], f32)
    gate_w = np.asarray(inputs["gate_w"], f32)
    gate_b = np.asarray(inputs["gate_b"], f32)

    a_k, a_q, a_v = att[:, :, :D], att[:, :, D:2 * D], att[:, :, 2 * D:]
    Wq_r = Wq.reshape(L, D, H, D)
    Wv_r = Wv.reshape(L, D, H, D)
    Wqa = np.ascontiguousarray(np.einsum("lkhd,lhd->lkh", Wq_r, a_q).astype(f32))
    Wva = np.ascontiguousarray(np.einsum("lkhd,lhd->lkh", Wv_r, a_v).astype(f32))
    c1 = np.einsum("ld,lhd->lh", Wk_w, a_k).astype(f32)       # [L,4]
    c0 = np.einsum("ld,lhd->lh", Wk_b, a_k).astype(f32)       # [L,4]
    gv1 = np.einsum("ld,lde->le", Wk_w, gate_w).astype(f32)   # [L,128]
    gv0 = (np.einsum("ld,lde->le", Wk_b, gate_w) + gate_b).astype(f32)

    def bc(v):  # broadcast a free-axis vector across 128 partitions
        return np.ascontiguousarray(np.tile(np.asarray(v, f32)[None, :], (128, 1)))

    common = {
        "Win": np.ascontiguousarray(np.asarray(inputs["W_in"], f32)),
        "binb": bc(inputs["b_in"]),
        "Wv3": np.ascontiguousarray(Wv),
        "Wqa3": Wqa, "Wva3": Wva,
        "c1b3": np.stack([bc(c1[l]) for l in range(L)]),
        "c0b3": np.stack([bc(c0[l]) for l in range(L)]),
        "Wo3": np.ascontiguousarray(
            np.asarray(inputs["Wo"], f32).reshape(L, 4, 128, 128)),
        "lngb3": np.stack([bc(np.asarray(inputs["ln_g"], f32)[l]) for l in range(L)]),
        "lnbb3": np.stack([bc(np.asarray(inputs["ln_b"], f32)[l]) for l in range(L)]),
        "blb3": np.stack([bc(np.asarray(inputs["b_l"], f32)[l]) for l in range(L)]),
        "gv1b3": np.stack([bc(gv1[l]) for l in range(L)]),
        "gv0b3": np.stack([bc(gv0[l]) for l in range(L)]),
        "Wout": np.ascontiguousarray(np.asarray(inputs["W_out"], f32)),
        "boutb": bc(inputs["b_out"]),
        "iota": np.ascontiguousarray(
            np.tile(np.arange(128, dtype=f32)[None, :], (128, 1))),
        "ident": np.eye(128, dtype=f32),
    }

    # xT: [128, NTAB] feature-major padded table layout
    xpad = np.zeros((NTAB, D), f32)
    for s in range(NCORES):
        xpad[s * SHARD_PAD:s * SHARD_PAD + SHARD] = x[s * SHARD:(s + 1) * SHARD]
    common["xT"] = np.ascontiguousarray(xpad.T)

    # ---- per-core edge batches ----
    tr_src = _trow(src)
    core_of = dst // SHARD
    ld_all = dst % SHARD

    tiles = [[None] * NT for _ in range(NCORES)]
    k1r = np.zeros((NCORES, NT), np.int64)
    k2r = np.zeros((NCORES, NT), np.int64)
    for c in range(NCORES):
        m = core_of == c
        sc, lc = tr_src[m], ld_all[m]
        order = np.argsort(lc, kind="stable")
        sc, lc = sc[order], lc[order]
        tile_id = lc // 128
        bounds = np.searchsorted(tile_id, np.arange(NT + 1))
        for t in range(NT):
            s_t = sc[bounds[t]:bounds[t + 1]]
            l_t = lc[bounds[t]:bounds[t + 1]]
            low = s_t < CHUNK
            tiles[c][t] = (s_t[low], s_t[~low] - CHUNK, l_t[low] % 128,
                           l_t[~low] % 128, l_t[low], l_t[~low])
            k1r[c, t] = int(low.sum())
            k2r[c, t] = int((~low).sum())

    K1 = (np.ceil(k1r.max(axis=0) / 128).astype(np.int64) * 128)
    K2 = (np.ceil(k2r.max(axis=0) / 128).astype(np.int64) * 128)
    C = (K1 + K2) // 128
    S = int(C.sum())
    meta = {"K1": K1.tolist(), "K2": K2.tolist(), "C": C.tolist(), "S": S}

    per_core = []
    for c in range(NCORES):
        src_cols, sd_cols = [], []
        rl = np.full((128, S), 200.0, np.float32)
        soff = 0
        for t in range(NT):
            lo_s, hi_s, lo_r, hi_r, lo_sd, hi_sd = tiles[c][t]
            k1, k2 = int(K1[t]), int(K2[t])
            seg_src = np.zeros(k1 + k2, np.int64)
            seg_src[:len(lo_s)] = lo_s
            seg_src[k1:k1 + len(hi_s)] = hi_s
            seg_sd = np.full(k1 + k2, SENT_ROW, np.int64)
            seg_sd[:len(lo_sd)] = lo_sd
            seg_sd[k1:k1 + len(hi_sd)] = hi_sd
            seg_r = np.full(k1 + k2, 200.0, np.float32)
            seg_r[:len(lo_r)] = lo_r
            seg_r[k1:k1 + len(hi_r)] = hi_r
            if k1:
                src_cols.append(_wrap_idx(seg_src[:k1]))
            if k2:
                src_cols.append(_wrap_idx(seg_src[k1:]))
            sd_cols.append(_wrap_idx(seg_sd))
            ct = (k1 + k2) // 128
            rl[:, soff:soff + ct] = seg_r.reshape(ct, 128).T
            soff += ct
        d = dict(common)
        d["srcidx"] = (np.ascontiguousarray(np.concatenate(src_cols, axis=1))
                       if src_cols else np.zeros((128, 16), np.int16))
        d["sdidx"] = np.ascontiguousarray(np.concatenate(sd_cols, axis=1))
        d["rloc"] = rl
        kk = np.zeros(SHARD_PAD, np.float32)
        kk[:SHARD] = K[c * SHARD:(c + 1) * SHARD]
        d["Ksb"] = np.ascontiguousarray(kk.reshape(NT, 128).T)
        per_core.append(d)
    meta["src_cols_total"] = per_core[0]["srcidx"].shape[1]
    meta["sd_cols_total"] = per_core[0]["sdidx"].shape[1]
    return meta, per_core


def _build(meta):
    from contextlib import ExitStack
    import concourse.bass as bass
    import concourse.tile as tile
    from concourse import bacc, mybir

    f32, bf16, i16 = mybir.dt.float32, mybir.dt.bfloat16, mybir.dt.int16
    AOP, AF = mybir.AluOpType, mybir.ActivationFunctionType
    K1, K2, C, S = meta["K1"], meta["K2"], meta["C"], meta["S"]

    STOP = os.environ.get("GNN_STOP", "full")  # h0 | 1b | l0 | l1 | full
    NOAG = bool(int(os.environ.get("GNN_NOAG", "0")))
    nc = bacc.Bacc("TRN2", target_bir_lowering=False, debug=False,
                   num_devices=NCORES)

    def inp(name, shape, dt=f32):
        return nc.dram_tensor(name, list(shape), dt, kind="ExternalInput")

    xT = inp("xT", [128, NTAB])
    Ksb_t = inp("Ksb", [128, NT])
    Win_t = inp("Win", [128, 128]); binb_t = inp("binb", [128, 128])
    Wv3_t = inp("Wv3", [L, 128, 512])
    Wqa3_t = inp("Wqa3", [L, 128, 4]); Wva3_t = inp("Wva3", [L, 128, 4])
    c1b3_t = inp("c1b3", [L, 128, 4]); c0b3_t = inp("c0b3", [L, 128, 4])
    Wo3_t = inp("Wo3", [L, 4, 128, 128])
    lngb3_t = inp("lngb3", [L, 128, 128]); lnbb3_t = inp("lnbb3", [L, 128, 128])
    blb3_t = inp("blb3", [L, 128, 128])
    gv1b3_t = inp("gv1b3", [L, 128, 128]); gv0b3_t = inp("gv0b3", [L, 128, 128])
    Wout_t = inp("Wout", [128, 64]); boutb_t = inp("boutb", [128, 64])
    iota_t = inp("iota", [128, 128]); ident_t = inp("ident", [128, 128])
    srcidx_t = inp("srcidx", [128, meta["src_cols_total"]], i16)
    sdidx_t = inp("sdidx", [128, meta["sd_cols_total"]], i16)
    rloc_t = inp("rloc", [128, S])
    out_t = nc.dram_tensor("out", [SHARD_PAD, 64], f32, kind="ExternalOutput")

    with tile.TileContext(nc) as tc, ExitStack() as ctx:
        dram = ctx.enter_context(tc.tile_pool(name="dram", bufs=1, space="DRAM"))
        pers = ctx.enter_context(tc.tile_pool(name="pers", bufs=1))
        wk = ctx.enter_context(tc.tile_pool(name="wk", bufs=2))
        wk3 = ctx.enter_context(tc.tile_pool(name="wk3", bufs=3))
        ps_v = ctx.enter_context(tc.tile_pool(name="ps_v", bufs=2, space="PSUM"))
        ps_s = ctx.enter_context(tc.tile_pool(name="ps_s", bufs=2, space="PSUM"))
        ps_u = ctx.enter_context(tc.tile_pool(name="ps_u", bufs=1, space="PSUM"))
        ps_m = ctx.enter_context(tc.tile_pool(name="ps_m", bufs=1, space="PSUM"))
        ps_d = ctx.enter_context(tc.tile_pool(name="ps_d", bufs=1, space="PSUM"))

        tabs = [dram.tile([NTAB, ROW], bf16, tag=f"tab{i}", name=f"tab{i}") for i in range(L)]
        shblks = [dram.tile([SHARD_PAD, ROW], bf16, tag=f"shblk{i}", name=f"shblk{i}")
                  for i in range(L - 1)]
        sdsts = [dram.tile([SD_ROWS, 64], f32, tag=f"sdst{i}", name=f"sdst{i}") for i in range(L)]

        # ---- persistent SBUF loads ----
        def load(tn, shape, dt=f32, tag=None):
            t = pers.tile(list(shape), dt, tag=tag or tn.name, name="sb_" + (tag or tn.name))
            nc.sync.dma_start(out=t[tuple(slice(None) for _ in shape)],
                              in_=tn[tuple(slice(None) for _ in shape)])
            return t

        Win = load(Win_t, [128, 128]); binb = load(binb_t, [128, 128])
        Wout = load(Wout_t, [128, 64]); boutb = load(boutb_t, [128, 64])
        iota = load(iota_t, [128, 128]); ident = load(ident_t, [128, 128])
        Ksb = load(Ksb_t, [128, NT])
        srcidx = load(srcidx_t, [128, meta["src_cols_total"]], i16)
        sdidx = load(sdidx_t, [128, meta["sd_cols_total"]], i16)
        rloc = load(rloc_t, [128, S])

        def ld3(tn, shape, tag, l):
            t = pers.tile(list(shape), f32, tag=f"{tag}{l}", name=f"sb_{tag}{l}")
            nc.sync.dma_start(out=t[tuple(slice(None) for _ in shape)], in_=tn[l])
            return t

        Wv_l = [ld3(Wv3_t, [128, 512], "Wv", l) for l in range(L)]
        Wqa_l = [ld3(Wqa3_t, [128, 4], "Wqa", l) for l in range(L)]
        Wva_l = [ld3(Wva3_t, [128, 4], "Wva", l) for l in range(L)]
        c1_l = [ld3(c1b3_t, [128, 4], "c1", l) for l in range(L)]
        c0_l = [ld3(c0b3_t, [128, 4], "c0", l) for l in range(L)]
        lng_l = [ld3(lngb3_t, [128, 128], "lng", l) for l in range(L)]
        lnb_l = [ld3(lnbb3_t, [128, 128], "lnb", l) for l in range(L)]
        bl_l = [ld3(blb3_t, [128, 128], "bl", l) for l in range(L)]
        gv1_l = [ld3(gv1b3_t, [128, 128], "gv1", l) for l in range(L)]
        gv0_l = [ld3(gv0b3_t, [128, 128], "gv0", l) for l in range(L)]
        Wo_l = []
        for l in range(L):
            row = []
            for kc in range(4):
                t = pers.tile([128, 128], f32, tag=f"Wo{l}_{kc}", name=f"sb_Wo{l}_{kc}")
                nc.sync.dma_start(out=t[:, :], in_=Wo3_t[l, kc])
                row.append(t)
            Wo_l.append(row)

        h_shard = [pers.tile([128, SHARD_PAD], f32, tag=f"hsh{i}", name=f"hsh{i}") for i in range(2)]

        # sentinel rows of each s_dst table
        epsb = pers.tile([128, 1], f32, tag="epsb", name="epsb")
        nc.vector.memset(epsb[:, :], LN_EPS)
        sent = pers.tile([128, 64], f32, tag="sent")
        nc.vector.memset(sent[:, :], -1e30)
        for l in range(L):
            nc.sync.dma_start(out=sdsts[l][SENT_ROW:SENT_ROW + 128, :],
                              in_=sent[:, :])

        def stage_rows(h_f32_ap):
            stag = wk.tile([128, ROW], bf16, tag="stag")
            nc.vector.tensor_copy(out=stag[:, 0:128], in_=h_f32_ap)
            if PLANES == 2:
                nc.vector.tensor_tensor(out=stag[:, 128:256], in0=h_f32_ap,
                                        in1=stag[:, 0:128], op=AOP.subtract)
            return stag

        MAGIC = 0x5f3759df

        def rsqrt_dve(out_ap, in_ap, shape, tmp_tag):
            # out = 1/sqrt(in) via quake seed + 2 Newton iters (all DVE)
            y = wk.tile(list(shape), f32, tag=f"{tmp_tag}_y")
            t_ = wk.tile(list(shape), f32, tag=f"{tmp_tag}_t")
            iy = y[tuple(slice(None) for _ in shape)].bitcast(mybir.dt.int32)
            iin = in_ap.bitcast(mybir.dt.int32)
            nc.vector.tensor_scalar(out=iy, in0=iin, scalar1=1,
                                    scalar2=-1, op0=AOP.arith_shift_right,
                                    op1=AOP.bitwise_xor)  # ~(i>>1)
            nc.vector.tensor_scalar(out=iy, in0=iy, scalar1=MAGIC + 1,
                                    scalar2=None, op0=AOP.add)
            ya = y[tuple(slice(None) for _ in shape)]
            ta = t_[tuple(slice(None) for _ in shape)]
            for _ in range(2):
                nc.vector.tensor_tensor(out=ta, in0=ya, in1=ya, op=AOP.mult)
                nc.vector.tensor_tensor(out=ta, in0=ta, in1=in_ap, op=AOP.mult)
                nc.vector.tensor_scalar(out=ta, in0=ta, scalar1=-0.5,
                                        scalar2=1.5, op0=AOP.mult, op1=AOP.add)
                nc.vector.tensor_tensor(out=ya, in0=ya, in1=ta, op=AOP.mult)
            nc.vector.tensor_copy(out=out_ap, in_=ya)

        def sdst_store(hT_sb, t, l):
            psd = ps_d.tile([128, 4], f32, tag="dps_a")
            nc.tensor.matmul(psd[:, :], lhsT=hT_sb[:, :], rhs=Wqa_l[l][:, :],
                             start=True, stop=True)
            sdt = wk.tile([128, 64], f32, tag="sdstag")
            nc.vector.memset(sdt[:, 4:64], 0.0)
            nc.vector.scalar_tensor_tensor(
                out=sdt[:, 0:4], in0=c1_l[l][:, :], scalar=Ksb[:, t:t + 1],
                in1=psd[:, :], op0=AOP.mult, op1=AOP.add)
            nc.vector.tensor_tensor(out=sdt[:, 0:4], in0=sdt[:, 0:4],
                                    in1=c0_l[l][:, :], op=AOP.add)
            nc.sync.dma_start(out=sdsts[l][t * 128:(t + 1) * 128, :],
                              in_=sdt[:, :])

        # ---- phase 1: replicated input projection -> tab0 (8-tile groups) ----
        NTT = NTAB // 128
        G1 = 8
        for t0 in range(0, NTT, G1):
            gn = min(G1, NTT - t0)
            xt = wk.tile([128, gn * 128], f32, tag="xt")
            nc.sync.dma_start(out=xt[:, :],
                              in_=xT[:, t0 * 128:(t0 + gn) * 128])
            sgrp = wk.tile([128, gn, ROW], bf16, tag="sgrp")
            for j in range(gn):
                ph = ps_d.tile([128, 128], f32, tag="dps_a")
                nc.tensor.matmul(ph[:, :], lhsT=xt[:, j * 128:(j + 1) * 128],
                                 rhs=Win[:, :], start=True, stop=True)
                h0 = wk3.tile([128, 128], f32, tag="h0")
                nc.vector.tensor_tensor(out=h0[:, :], in0=ph[:, :],
                                        in1=binb[:, :], op=AOP.add)
                nc.vector.tensor_scalar(out=h0[:, :], in0=h0[:, :], scalar1=0.0,
                                        scalar2=None, op0=AOP.max)
                nc.vector.tensor_copy(out=sgrp[:, j, 0:128], in_=h0[:, :])
                if PLANES == 2:
                    nc.vector.tensor_tensor(out=sgrp[:, j, 128:256],
                                            in0=h0[:, :], in1=sgrp[:, j, 0:128],
                                            op=AOP.subtract)
            nc.sync.dma_start(out=tabs[0][t0 * 128:(t0 + gn) * 128, :],
                              in_=sgrp[:, :, :])

        # ---- phase 1b: own-shard h0 capture (dynamic row offset) ----
        pid = None
        if STOP != "h0":
            pid = nc.sync.partition_id()
        row0 = (pid * SHARD_PAD) if pid is not None else None
        for t in (range(NT) if STOP != "h0" else []):
            g = wk.tile([128, ROW], bf16, tag="g1b")
            nc.sync.dma_start(out=g[:, :],
                              in_=tabs[0][bass.ds(row0 + t * 128, 128), :])
            hs = h_shard[0][:, t * 128:(t + 1) * 128]
            if PLANES == 2:
                nc.vector.tensor_tensor(out=hs, in0=g[:, 0:128],
                                        in1=g[:, 128:256], op=AOP.add)
            else:
                nc.vector.tensor_copy(out=hs, in_=g[:, 0:128])
            ptr = ps_d.tile([128, 128], f32, tag="dps_a")
            nc.tensor.transpose(ptr[:, :], hs, ident[:, :])
            hT = wk.tile([128, 128], f32, tag="hT1b")
            nc.vector.tensor_copy(out=hT[:, :], in_=ptr[:, :])
            sdst_store(hT, t, 0)

        # ---- per-layer gate table buffer: g = 1/(1+exp(-(K*gv1+gv0))) ----
        gate_buf = pers.tile([128, SHARD_PAD], f32, tag="gate", name="gate")

        def build_gate(l):
            for t in range(NT):
                nc.vector.scalar_tensor_tensor(
                    out=gate_buf[:, t * 128:(t + 1) * 128], in0=gv1_l[l][:, :],
                    scalar=Ksb[:, t:t + 1], in1=gv0_l[l][:, :],
                    op0=AOP.mult, op1=AOP.add)
            nc.scalar.activation(out=gate_buf[:, :], in_=gate_buf[:, :],
                                 func=AF.Exp, scale=-1.0)
            nc.vector.tensor_scalar(out=gate_buf[:, :], in0=gate_buf[:, :],
                                    scalar1=1.0, scalar2=None, op0=AOP.add)
            nc.vector.reciprocal(out=gate_buf[:, :], in_=gate_buf[:, :])

        # ---- layers ----
        n_layers = {"h0": 0, "1b": 0, "l0": 1, "l1": 2}.get(STOP, L)
        for l in range(n_layers):
            tab = tabs[l]
            h_cur = h_shard[l % 2]
            h_nxt = h_shard[(l + 1) % 2]
            build_gate(l)
            soff = 0
            co_src = 0
            co_sd = 0
            for t in range(NT):
                k1, k2, ct = K1[t], K2[t], C[t]
                # gathers split into <=GCAP-idx calls (SWDGE ring limit ~1024)
                GCAP = 768
                gl_chunks, gh_chunks = [], []
                for off in range(0, k1, GCAP):
                    n = min(GCAP, k1 - off)
                    gt_ = wk.tile([128, PLANES, n], bf16, tag=f"glow{off//GCAP}")
                    nc.gpsimd.dma_gather(
                        out_ap=gt_[:, :, :], in_ap=tab[0:CHUNK, :],
                        idxs_ap=srcidx[:, co_src:co_src + n // 16],
                        num_idxs=n, num_idxs_reg=n, elem_size=ROW,
                        transpose=True)
                    co_src += n // 16
                    gl_chunks.append((off, n, gt_))
                for off in range(0, k2, GCAP):
                    n = min(GCAP, k2 - off)
                    gt_ = wk.tile([128, PLANES, n], bf16, tag=f"ghigh{off//GCAP}")
                    nc.gpsimd.dma_gather(
                        out_ap=gt_[:, :, :], in_ap=tab[CHUNK:NTAB, :],
                        idxs_ap=srcidx[:, co_src:co_src + n // 16],
                        num_idxs=n, num_idxs_reg=n, elem_size=ROW,
                        transpose=True)
                    co_src += n // 16
                    gh_chunks.append((k1 + off, n, gt_))
                SDCAP = 6  # slots per sd gather call (<=768 idxs)
                sd_chunks = []
                for c0 in range(0, ct, SDCAP):
                    nsl = min(SDCAP, ct - c0)
                    sdt_ = wk.tile([128, nsl, 64], f32, tag=f"sd{c0//SDCAP}")
                    nc.gpsimd.dma_gather(
                        out_ap=sdt_[:, :, :], in_ap=sdsts[l][:, :],
                        idxs_ap=sdidx[:, co_sd:co_sd + nsl * 8],
                        num_idxs=nsl * 128, num_idxs_reg=nsl * 128, elem_size=64,
                        transpose=False)
                    co_sd += nsl * 8
                    sd_chunks.append(sdt_)
                hsrc = wk.tile([128, ct * 128], f32, tag="hsrc")
                for off, n, gt_ in gl_chunks + gh_chunks:
                    if PLANES == 2:
                        nc.vector.tensor_tensor(out=hsrc[:, off:off + n],
                                                in0=gt_[:, 0, :], in1=gt_[:, 1, :],
                                                op=AOP.add)
                    else:
                        nc.vector.tensor_copy(out=hsrc[:, off:off + n],
                                              in_=gt_[:, 0, :])

                # pass 1: s_src matmuls -> batched scores -> one Exp
                sca = wk.tile([128, ct * 4], f32, tag="sca")
                for c in range(ct):
                    sl = hsrc[:, c * 128:(c + 1) * 128]
                    pss = ps_s.tile([128, 4], f32, tag="pss")
                    nc.tensor.matmul(pss[:, :], lhsT=sl, rhs=Wva_l[l][:, :],
                                     start=True, stop=True)
                    sd_ch = sd_chunks[c // SDCAP]
                    nc.vector.tensor_tensor(out=sca[:, c * 4:(c + 1) * 4],
                                            in0=pss[:, :],
                                            in1=sd_ch[:, c % SDCAP, 0:4],
                                            op=AOP.add)
                exa = wk.tile([128, ct * 4], f32, tag="exa")
                nc.vector.scalar_tensor_tensor(
                    out=exa[:, :], in0=sca[:, :], scalar=NEG_SLOPE,
                    in1=sca[:, :], op0=AOP.mult, op1=AOP.max)
                nc.scalar.activation(out=exa[:, :], in_=exa[:, :], func=AF.Exp)
                # pass 2: v matmuls, messages, indicator aggregation
                pu = ps_u.tile([128, 512], f32, tag="pu")
                pm = ps_m.tile([128, 4], f32, tag="pm")
                for c in range(ct):
                    sl = hsrc[:, c * 128:(c + 1) * 128]
                    pv = ps_v.tile([128, 512], f32, tag="pv")
                    nc.tensor.matmul(pv[:, :], lhsT=sl, rhs=Wv_l[l][:, :],
                                     start=True, stop=True)
                    M = wk3.tile([128, 512], f32, tag="M")
                    exb = exa[:, c * 4:(c + 1) * 4].rearrange(
                        "p (h o) -> p h o", o=1).broadcast_to([128, 4, 128])
                    nc.vector.tensor_tensor(
                        out=M[:, :].rearrange("p (h d) -> p h d", h=4),
                        in0=pv[:, :].rearrange("p (h d) -> p h d", h=4),
                        in1=exb, op=AOP.mult)
                    ind = wk3.tile([128, 128], f32, tag="ind")
                    nc.gpsimd.tensor_scalar(
                        out=ind[:, :], in0=iota[:, :],
                        scalar1=rloc[:, soff + c:soff + c + 1],
                        scalar2=None, op0=AOP.is_equal)
                    nc.tensor.matmul(pu[:, :], lhsT=ind[:, :], rhs=M[:, :],
                                     start=(c == 0), stop=(c == ct - 1))
                    nc.tensor.matmul(pm[:, :], lhsT=ind[:, :],
                                     rhs=exa[:, c * 4:(c + 1) * 4],
                                     start=(c == 0), stop=(c == ct - 1))
                soff += ct

                # -- normalize agg = U / sm --
                smr = wk.tile([128, 4], f32, tag="smr")
                nc.vector.tensor_scalar(out=smr[:, :], in0=pm[:, :], scalar1=1e-30,
                                        scalar2=None, op0=AOP.max)
                nc.vector.reciprocal(out=smr[:, :], in_=smr[:, :])
                agg = wk.tile([128, 512], f32, tag="agg")
                smb = smr[:, :].rearrange("p (h o) -> p h o", o=1).broadcast_to(
                    [128, 4, 128])
                nc.vector.tensor_tensor(
                    out=agg[:, :].rearrange("p (h d) -> p h d", h=4),
                    in0=pu[:, :].rearrange("p (h d) -> p h d", h=4),
                    in1=smb, op=AOP.mult)
                po = ps_d.tile([128, 128], f32, tag="dps_b")
                for kc in range(4):
                    ptr = ps_d.tile([128, 128], f32, tag="dps_a")
                    nc.tensor.transpose(ptr[:, :], agg[:, kc * 128:(kc + 1) * 128],
                                        ident[:, :])
                    aT = wk3.tile([128, 128], f32, tag="aT")
                    nc.vector.tensor_copy(out=aT[:, :], in_=ptr[:, :])
                    nc.tensor.matmul(po[:, :], lhsT=aT[:, :], rhs=Wo_l[l][kc][:, :],
                                     start=(kc == 0), stop=(kc == 3))
                res = wk.tile([128, 128], f32, tag="res")
                nc.vector.tensor_tensor(out=res[:, :], in0=po[:, :],
                                        in1=h_cur[:, t * 128:(t + 1) * 128],
                                        op=AOP.add)
                # -- LN --
                mu = wk.tile([128, 1], f32, tag="mu")
                nc.vector.tensor_reduce(out=mu[:, :], in_=res[:, :],
                                        axis=mybir.AxisListType.X, op=AOP.add)
                nc.vector.tensor_scalar(out=mu[:, :], in0=mu[:, :],
                                        scalar1=1.0 / 128, scalar2=None,
                                        op0=AOP.mult)
                xc = wk.tile([128, 128], f32, tag="xc")
                nc.vector.tensor_scalar(out=xc[:, :], in0=res[:, :],
                                        scalar1=mu[:, :], scalar2=None,
                                        op0=AOP.subtract)
                sq = wk.tile([128, 128], f32, tag="sq")
                vsum = wk.tile([128, 1], f32, tag="vsum")
                nc.vector.scalar_tensor_tensor(out=sq[:, :], in0=xc[:, :],
                                               scalar=1.0, in1=xc[:, :],
                                               op0=AOP.mult, op1=AOP.mult,
                                               accum_out=vsum[:, :])
                nc.vector.tensor_scalar(out=vsum[:, :], in0=vsum[:, :],
                                        scalar1=1.0 / 128, scalar2=LN_EPS,
                                        op0=AOP.mult, op1=AOP.add)
                rstd = wk.tile([128, 1], f32, tag="rstd")
                rsqrt_dve(rstd[:, :], vsum[:, :], [128, 1], "rsq")
                t1 = wk.tile([128, 128], f32, tag="t1")
                nc.vector.scalar_tensor_tensor(out=t1[:, :], in0=xc[:, :],
                                               scalar=rstd[:, :],
                                               in1=lng_l[l][:, :],
                                               op0=AOP.mult, op1=AOP.mult)
                nc.vector.tensor_tensor(out=t1[:, :], in0=t1[:, :],
                                        in1=lnb_l[l][:, :], op=AOP.add)
                # gate (precomputed) and combine
                gt = gate_buf[:, t * 128:(t + 1) * 128]
                m1 = wk.tile([128, 128], f32, tag="m1")
                nc.vector.tensor_tensor(out=m1[:, :], in0=t1[:, :], in1=gt,
                                        op=AOP.mult)
                nc.vector.tensor_tensor(out=m1[:, :], in0=m1[:, :], in1=t1[:, :],
                                        op=AOP.add)
                nc.vector.tensor_tensor(out=m1[:, :], in0=m1[:, :],
                                        in1=bl_l[l][:, :], op=AOP.add)
                hn = h_nxt[:, t * 128:(t + 1) * 128]
                nc.vector.tensor_scalar(out=hn, in0=m1[:, :], scalar1=0.0,
                                        scalar2=None, op0=AOP.max)

                ptr2 = ps_d.tile([128, 128], f32, tag="dps_a")
                nc.tensor.transpose(ptr2[:, :], hn, ident[:, :])
                hT = wk3.tile([128, 128], f32, tag="hTn")
                nc.vector.tensor_copy(out=hT[:, :], in_=ptr2[:, :])
                if l < L - 1:
                    sdst_store(hT, t, l + 1)
                    stag = stage_rows(hn)
                    nc.sync.dma_start(
                        out=shblks[l][t * 128:(t + 1) * 128, :], in_=stag[:, :])
                else:
                    pf = ps_d.tile([128, 64], f32, tag="dps_a")
                    nc.tensor.matmul(pf[:, :], lhsT=hT[:, :], rhs=Wout[:, :],
                                     start=True, stop=True)
                    ob = wk.tile([128, 64], f32, tag="ob")
                    nc.vector.tensor_tensor(out=ob[:, :], in0=pf[:, :],
                                            in1=boutb[:, :], op=AOP.add)
                    nc.sync.dma_start(out=out_t[t * 128:(t + 1) * 128, :],
                                      in_=ob[:, :])
            if l < L - 1 and not NOAG:
                nc.gpsimd.collective_compute(
                    "AllGather", mybir.AluOpType.bypass,
                    replica_groups=[list(range(NCORES))],
                    ins=[shblks[l].opt()],
                    outs=[tabs[l + 1].opt()],
                )

    nc.compile()
    return nc


def kernel(**inputs):
    from concourse import bass_utils

    meta, per_core = _prep(inputs)
    if "nc" not in _CACHE:
        _CACHE["nc"] = _build(meta)
    nc = _CACHE["nc"]
    res = bass_utils.run_bass_kernel_spmd(nc, per_core,
                                          core_ids=list(range(NCORES)))
    out = np.concatenate(
        [res.results[c]["out"][:SHARD] for c in range(NCORES)], axis=0)
    return np.ascontiguousarray(out.astype(np.float32))


